# revision 2
# baseline (speedup 1.0000x reference)
"""Mixtral-style MoE block (T=2048, H=1024, F=2048, E=8, top-2) on 8 trn2
NeuronCores.

Expert-parallel with host-side dispatch: the router (a [2048,8] matmul +
softmax + top-2) is computed on host in fp32, and each core receives only
the tokens routed to its expert, capacity-padded to C (= max expert load
rounded up). The core computes its expert's SwiGLU FFN in bf16 (fp32 PSUM
accumulation), scales rows by the renormalized top-2 combine weight, and
writes its [C, H] partial; the host scatter-adds the two partials per
token. No collectives. bf16 halves both Tensor rows and HBM bytes vs
fp32r; sparse dispatch cuts matmul work 2048/C (~3.6x) vs dense.
"""
import numpy as np
import ml_dtypes

try:
    import concourse  # noqa: F401
except ImportError:  # pragma: no cover
    import sys
    sys.path.insert(0, "/opt/trn_rl_repo")

from concourse import mybir, bacc
import concourse.tile as tile
from concourse.bass_utils import run_bass_kernel_spmd

T, H, F, E, TOP_K = 2048, 1024, 2048, 8, 2
P = 128
KH = H // P          # 8 k-tiles over H
KF = F // P          # 16 f-tiles over F
F32 = mybir.dt.float32
BF16 = mybir.dt.bfloat16
BF16NP = ml_dtypes.bfloat16
PSUM = "PSUM"

_NC_CACHE = {}


def _route(hidden_states, gate_w):
    """Host router, replicating reference: softmax fp32 -> top-2 (ties to
    lower index, like lax.top_k) -> renormalize."""
    x = np.asarray(hidden_states, np.float32)
    logits = x @ np.asarray(gate_w, np.float32)
    logits = logits - logits.max(-1, keepdims=True)
    p = np.exp(logits)
    p /= p.sum(-1, keepdims=True)
    idx = np.argsort(-p, axis=-1, kind="stable")[:, :TOP_K]
    tw = np.take_along_axis(p, idx, -1)
    tw = (tw / tw.sum(-1, keepdims=True)).astype(np.float32)
    toks, wts = [], []
    for e in range(E):
        sel = idx == e                      # [T, K]; <=1 hit per token
        t = np.nonzero(sel.any(1))[0]
        w = np.where(sel[:, 0], tw[:, 0], tw[:, 1])[t]
        toks.append(t)
        wts.append(w.astype(np.float32))
    cap = max(len(t) for t in toks)
    C = max(((cap + 31) // 32) * 32, P)     # capacity, 32-aligned
    return toks, wts, C


def _chunks(C):
    """Split C tokens into equal-ish 32-aligned PSUM chunks of <=512."""
    n = -(-C // 512)
    s = ((-(-C // n) + 31) // 32) * 32
    bounds = []
    c0 = 0
    while c0 < C:
        c1 = min(c0 + s, C)
        bounds.append((c0, c1))
        c0 = c1
    return s, bounds


def build(C):
    MT = -(-C // P)                     # phase-B m-tiles (last may be partial)
    MS = [P] * (C // P) + ([C % P] if C % P else [])
    s, chunks = _chunks(C)

    nc = bacc.Bacc("TRN2", target_bir_lowering=False, debug=False,
                   num_devices=E)
    xg = nc.dram_tensor("xg", [P, KH, C], BF16, kind="ExternalInput")
    w13 = nc.dram_tensor("w13", [P, KF, 2, KH, P], BF16, kind="ExternalInput")
    w2d = nc.dram_tensor("w2d", [P, KF, H], BF16, kind="ExternalInput")
    cwd = nc.dram_tensor("cwd", [P, MT], F32, kind="ExternalInput")
    oute = nc.dram_tensor("oute", [C, H], F32, kind="ExternalOutput")

    with tile.TileContext(nc) as tc:
        with (
            tc.tile_pool(name="big", bufs=1) as big,
            tc.tile_pool(name="small", bufs=1) as small,
            tc.tile_pool(name="wpool", bufs=3) as wpool,
            tc.tile_pool(name="evac", bufs=4) as evac,
        ):
            # token shards: one tile per PSUM chunk so compute starts on
            # chunk 0 as soon as its DMA lands
            xg_s = []
            for i, (c0, c1) in enumerate(chunks):
                xt = big.tile([P, KH, c1 - c0], BF16, name=f"xg{i}")
                eng = nc.gpsimd if i == 0 else nc.sync
                eng.dma_start(out=xt[:], in_=xg.ap()[:, :, c0:c1])
                xg_s.append(xt)
            cw_s = small.tile([P, MT], F32)
            nc.sync.dma_start(out=cw_s[:], in_=cwd.ap())
            # w2 resident in SBUF (4MB bf16), streamed in under phase A
            w2_s = big.tile([P, KF, H], BF16)
            nc.sync.dma_start(out=w2_s[:], in_=w2d.ap())
            inter = big.tile([P, KF, C], BF16)  # inter[f%P, f//P, tok]

            # Phase A: inter[f, t] = silu(w1.T x)[f, t] * (w3.T x)[f, t]
            with tc.tile_pool(name="psA", bufs=2, space=PSUM) as psA:
                for f in range(KF):
                    wf = wpool.tile([P, 2, KH, P], BF16, tag="wf", name="wf",
                                    bufs=3)
                    nc.gpsimd.dma_start(out=wf[:], in_=w13.ap()[:, f])
                    for i, (c0, c1) in enumerate(chunks):
                        w = c1 - c0
                        ps1 = psA.tile([P, s], F32, tag="ps1", name="ps1")
                        for k in range(KH):
                            nc.tensor.matmul(ps1[:, :w], lhsT=wf[:, 0, k, :],
                                             rhs=xg_s[i][:, k, :],
                                             start=(k == 0), stop=(k == KH - 1))
                        ps3 = psA.tile([P, s], F32, tag="ps3", name="ps3")
                        for k in range(KH):
                            nc.tensor.matmul(ps3[:, :w], lhsT=wf[:, 1, k, :],
                                             rhs=xg_s[i][:, k, :],
                                             start=(k == 0), stop=(k == KH - 1))
                        sil = evac.tile([P, s], F32, tag="sil", name="sil")
                        nc.scalar.activation(sil[:, :w], ps1[:, :w],
                                             mybir.ActivationFunctionType.Silu)
                        nc.vector.tensor_tensor(inter[:, f, c0:c1],
                                                sil[:, :w], ps3[:, :w],
                                                op=mybir.AluOpType.mult)

            # Phase B: oute[t, :] = cw[t] * (inter.T @ w2)[t, :]
            with tc.tile_pool(name="psB", bufs=1, space=PSUM) as psB:
                for n in range(H // 512):
                    pss = [psB.tile([P, 512], F32, tag=f"psb{m}",
                                    name=f"psb{m}") for m in range(MT)]
                    for k in range(KF):
                        for m, ms in enumerate(MS):
                            nc.tensor.matmul(
                                pss[m][:ms, :],
                                lhsT=inter[:, k, m * P:m * P + ms],
                                rhs=w2_s[:, k, n * 512:(n + 1) * 512],
                                start=(k == 0), stop=(k == KF - 1))
                    for m, ms in enumerate(MS):
                        o = evac.tile([P, 512], F32, tag="o", name="o")
                        nc.vector.tensor_scalar_mul(o[:ms, :], pss[m][:ms, :],
                                                    cw_s[:ms, m:m + 1])
                        nc.sync.dma_start(
                            out=oute.ap()[m * P:m * P + ms,
                                          n * 512:(n + 1) * 512],
                            in_=o[:ms, :])
    nc.compile()
    return nc


def kernel(hidden_states, gate_w, w1, w2, w3):
    in_maps = make_in_maps(hidden_states, gate_w, w1, w2, w3)
    nc = _NC_CACHE["nc"]
    res = run_bass_kernel_spmd(nc, in_maps, core_ids=list(range(E)),
                               trace=False)
    return assemble(res.results)


def make_in_maps(hidden_states, gate_w, w1, w2, w3):
    toks, wts, C = _route(hidden_states, gate_w)
    _NC_CACHE["route"] = (toks, wts, C)
    if "nc" not in _NC_CACHE or _NC_CACHE.get("C") != C:
        _NC_CACHE["nc"] = build(C)
        _NC_CACHE["C"] = C
    MT = -(-C // P)
    x = np.asarray(hidden_states, np.float32)
    in_maps = []
    for e in range(E):
        n_e = len(toks[e])
        xpad = np.zeros((C, H), np.float32)
        xpad[:n_e] = x[toks[e]]
        xgn = np.ascontiguousarray(
            xpad.reshape(C, KH, P).transpose(2, 1, 0).astype(BF16NP))
        w1r = np.asarray(w1[e], np.float32).reshape(KH, P, KF, P)
        w3r = np.asarray(w3[e], np.float32).reshape(KH, P, KF, P)
        w13n = np.ascontiguousarray(
            np.stack([w1r, w3r], 0).transpose(2, 3, 0, 1, 4).astype(BF16NP))
        w2n = np.ascontiguousarray(
            np.asarray(w2[e], np.float32).reshape(KF, P, H)
            .transpose(1, 0, 2).astype(BF16NP))
        wpad = np.zeros(MT * P, np.float32)
        wpad[:n_e] = wts[e]
        cwn = np.ascontiguousarray(wpad.reshape(MT, P).T)
        in_maps.append({"xg": xgn, "w13": w13n, "w2d": w2n, "cwd": cwn})
    return in_maps


def assemble(results):
    toks, _, _ = _NC_CACHE["route"]
    out = np.zeros((T, H), np.float32)
    for e in range(E):
        out[toks[e]] += results[e]["oute"][:len(toks[e])]
    return out


# revision 4
# speedup vs baseline: 1.1473x; 1.1473x over previous
"""Mixtral-style MoE block (T=2048, H=1024, F=2048, E=8, top-2) on 8 trn2
NeuronCores.

Expert-parallel with host-side dispatch: the router (a [2048,8] matmul +
softmax + top-2) is computed on host in fp32, and each core receives only
the tokens routed to its expert, capacity-padded to C (= max expert load
rounded up). The core computes its expert's SwiGLU FFN in bf16 (fp32 PSUM
accumulation), scales rows by the renormalized top-2 combine weight, and
writes its [C, H] partial; the host scatter-adds the two partials per
token. No collectives. bf16 halves both Tensor rows and HBM bytes vs
fp32r; sparse dispatch cuts matmul work 2048/C (~3.6x) vs dense.
"""
import numpy as np
import ml_dtypes

try:
    import concourse  # noqa: F401
except ImportError:  # pragma: no cover
    import sys
    sys.path.insert(0, "/opt/trn_rl_repo")

from concourse import mybir, bacc
import concourse.tile as tile
from concourse.bass_utils import run_bass_kernel_spmd

T, H, F, E, TOP_K = 2048, 1024, 2048, 8, 2
P = 128
KH = H // P          # 8 k-tiles over H
KF = F // P          # 16 f-tiles over F
F32 = mybir.dt.float32
BF16 = mybir.dt.bfloat16
BF16NP = ml_dtypes.bfloat16
PSUM = "PSUM"

_NC_CACHE = {}


def _route(hidden_states, gate_w):
    """Host router, replicating reference: softmax fp32 -> top-2 (ties to
    lower index, like lax.top_k) -> renormalize."""
    x = np.asarray(hidden_states, np.float32)
    logits = x @ np.asarray(gate_w, np.float32)
    logits = logits - logits.max(-1, keepdims=True)
    p = np.exp(logits)
    p /= p.sum(-1, keepdims=True)
    idx = np.argsort(-p, axis=-1, kind="stable")[:, :TOP_K]
    tw = np.take_along_axis(p, idx, -1)
    tw = (tw / tw.sum(-1, keepdims=True)).astype(np.float32)
    toks, wts = [], []
    for e in range(E):
        sel = idx == e                      # [T, K]; <=1 hit per token
        t = np.nonzero(sel.any(1))[0]
        w = np.where(sel[:, 0], tw[:, 0], tw[:, 1])[t]
        toks.append(t)
        wts.append(w.astype(np.float32))
    cap = max(len(t) for t in toks)
    C = max(((cap + 31) // 32) * 32, P)     # capacity, 32-aligned
    return toks, wts, C


def _chunks(C):
    """Split C tokens into equal-ish 32-aligned PSUM chunks of <=512."""
    n = -(-C // 512)
    s = ((-(-C // n) + 31) // 32) * 32
    bounds = []
    c0 = 0
    while c0 < C:
        c1 = min(c0 + s, C)
        bounds.append((c0, c1))
        c0 = c1
    return s, bounds


def build(C):
    MT = -(-C // P)                     # phase-B m-tiles (last may be partial)
    MS = [P] * (C // P) + ([C % P] if C % P else [])
    s, chunks = _chunks(C)

    nc = bacc.Bacc("TRN2", target_bir_lowering=False, debug=False,
                   num_devices=E)
    xg = nc.dram_tensor("xg", [P, KH, C], BF16, kind="ExternalInput")
    w13 = nc.dram_tensor("w13", [P, KF, 2, KH, P], BF16, kind="ExternalInput")
    w2d = nc.dram_tensor("w2d", [P, KF, H], BF16, kind="ExternalInput")
    cwd = nc.dram_tensor("cwd", [P, MT], F32, kind="ExternalInput")
    oute = nc.dram_tensor("oute", [C, H], F32, kind="ExternalOutput")

    with tile.TileContext(nc) as tc:
        with (
            tc.tile_pool(name="big", bufs=1) as big,
            tc.tile_pool(name="small", bufs=1) as small,
            tc.tile_pool(name="wpool", bufs=3) as wpool,
            tc.tile_pool(name="evac", bufs=4) as evac,
        ):
            # token shards: one tile per PSUM chunk so compute starts on
            # chunk 0 as soon as its DMA lands
            xg_s = []
            for i, (c0, c1) in enumerate(chunks):
                xt = big.tile([P, KH, c1 - c0], BF16, name=f"xg{i}")
                eng = nc.gpsimd if i == 0 else nc.sync
                eng.dma_start(out=xt[:], in_=xg.ap()[:, :, c0:c1])
                xg_s.append(xt)
            cw_s = small.tile([P, MT], F32)
            nc.sync.dma_start(out=cw_s[:], in_=cwd.ap())
            # w2 resident in SBUF (4MB bf16); streamed per k-slice inside the
            # phase A loop so it does not contend with the first wf/xg DMAs
            w2_s = big.tile([P, KF, H], BF16)
            inter = big.tile([P, KF, C], BF16)  # inter[f%P, f//P, tok]

            # Phase A: inter[f, t] = silu(w1.T x)[f, t] * (w3.T x)[f, t]
            with tc.tile_pool(name="psA", bufs=2, space=PSUM) as psA:
                for f in range(KF):
                    wf = wpool.tile([P, 2, KH, P], BF16, tag="wf", name="wf",
                                    bufs=3)
                    nc.gpsimd.dma_start(out=wf[:], in_=w13.ap()[:, f])
                    nc.sync.dma_start(out=w2_s[:, f, :], in_=w2d.ap()[:, f, :])
                    for i, (c0, c1) in enumerate(chunks):
                        w = c1 - c0
                        ps1 = psA.tile([P, s], F32, tag="ps1", name="ps1")
                        for k in range(KH):
                            nc.tensor.matmul(ps1[:, :w], lhsT=wf[:, 0, k, :],
                                             rhs=xg_s[i][:, k, :],
                                             start=(k == 0), stop=(k == KH - 1))
                        ps3 = psA.tile([P, s], F32, tag="ps3", name="ps3")
                        for k in range(KH):
                            nc.tensor.matmul(ps3[:, :w], lhsT=wf[:, 1, k, :],
                                             rhs=xg_s[i][:, k, :],
                                             start=(k == 0), stop=(k == KH - 1))
                        sil = evac.tile([P, s], F32, tag="sil", name="sil")
                        nc.scalar.activation(sil[:, :w], ps1[:, :w],
                                             mybir.ActivationFunctionType.Silu)
                        nc.vector.tensor_tensor(inter[:, f, c0:c1],
                                                sil[:, :w], ps3[:, :w],
                                                op=mybir.AluOpType.mult)

            # Phase B: oute[t, :] = cw[t] * (inter.T @ w2)[t, :]
            # m-outer so PSUM (2 banks per m-tile, double-buffered) frees as
            # soon as that m-tile's evac runs -- no barrier between m-tiles
            with tc.tile_pool(name="psB", bufs=2, space=PSUM) as psB:
                for m, ms in enumerate(MS):
                    pss = [psB.tile([P, 512], F32, tag=f"psb{n}",
                                    name=f"psb{n}") for n in range(H // 512)]
                    for k in range(KF):
                        for n in range(H // 512):
                            nc.tensor.matmul(
                                pss[n][:ms, :],
                                lhsT=inter[:, k, m * P:m * P + ms],
                                rhs=w2_s[:, k, n * 512:(n + 1) * 512],
                                start=(k == 0), stop=(k == KF - 1))
                    for n in range(H // 512):
                        o = evac.tile([P, 512], F32, tag="o", name="o")
                        nc.vector.tensor_scalar_mul(o[:ms, :], pss[n][:ms, :],
                                                    cw_s[:ms, m:m + 1])
                        nc.sync.dma_start(
                            out=oute.ap()[m * P:m * P + ms,
                                          n * 512:(n + 1) * 512],
                            in_=o[:ms, :])
    nc.compile()
    return nc


def kernel(hidden_states, gate_w, w1, w2, w3):
    in_maps = make_in_maps(hidden_states, gate_w, w1, w2, w3)
    nc = _NC_CACHE["nc"]
    res = run_bass_kernel_spmd(nc, in_maps, core_ids=list(range(E)),
                               trace=False)
    return assemble(res.results)


def make_in_maps(hidden_states, gate_w, w1, w2, w3):
    toks, wts, C = _route(hidden_states, gate_w)
    _NC_CACHE["route"] = (toks, wts, C)
    if "nc" not in _NC_CACHE or _NC_CACHE.get("C") != C:
        _NC_CACHE["nc"] = build(C)
        _NC_CACHE["C"] = C
    MT = -(-C // P)
    x = np.asarray(hidden_states, np.float32)
    in_maps = []
    for e in range(E):
        n_e = len(toks[e])
        xpad = np.zeros((C, H), np.float32)
        xpad[:n_e] = x[toks[e]]
        xgn = np.ascontiguousarray(
            xpad.reshape(C, KH, P).transpose(2, 1, 0).astype(BF16NP))
        w1r = np.asarray(w1[e], np.float32).reshape(KH, P, KF, P)
        w3r = np.asarray(w3[e], np.float32).reshape(KH, P, KF, P)
        w13n = np.ascontiguousarray(
            np.stack([w1r, w3r], 0).transpose(2, 3, 0, 1, 4).astype(BF16NP))
        w2n = np.ascontiguousarray(
            np.asarray(w2[e], np.float32).reshape(KF, P, H)
            .transpose(1, 0, 2).astype(BF16NP))
        wpad = np.zeros(MT * P, np.float32)
        wpad[:n_e] = wts[e]
        cwn = np.ascontiguousarray(wpad.reshape(MT, P).T)
        in_maps.append({"xg": xgn, "w13": w13n, "w2d": w2n, "cwd": cwn})
    return in_maps


def assemble(results):
    toks, _, _ = _NC_CACHE["route"]
    out = np.zeros((T, H), np.float32)
    for e in range(E):
        out[toks[e]] += results[e]["oute"][:len(toks[e])]
    return out


# revision 8
# speedup vs baseline: 1.1495x; 1.0020x over previous
"""Mixtral-style MoE block (T=2048, H=1024, F=2048, E=8, top-2) on 8 trn2
NeuronCores.

Expert-parallel with host-side dispatch: the router (a [2048,8] matmul +
softmax + top-2) is computed on host in fp32, and each core receives only
the tokens routed to its expert, capacity-padded to C (= max expert load
rounded up). The core computes its expert's SwiGLU FFN in bf16 (fp32 PSUM
accumulation), scales rows by the renormalized top-2 combine weight, and
writes its [C, H] partial; the host scatter-adds the two partials per
token. No collectives. bf16 halves both Tensor rows and HBM bytes vs
fp32r; sparse dispatch cuts matmul work 2048/C (~3.6x) vs dense.
"""
import numpy as np
import ml_dtypes

try:
    import concourse  # noqa: F401
except ImportError:  # pragma: no cover
    import sys
    sys.path.insert(0, "/opt/trn_rl_repo")

from concourse import mybir, bacc
import concourse.tile as tile
from concourse.bass_utils import run_bass_kernel_spmd

T, H, F, E, TOP_K = 2048, 1024, 2048, 8, 2
P = 128
KH = H // P          # 8 k-tiles over H
KF = F // P          # 16 f-tiles over F
F32 = mybir.dt.float32
BF16 = mybir.dt.bfloat16
BF16NP = ml_dtypes.bfloat16
PSUM = "PSUM"

_NC_CACHE = {}


def _route(hidden_states, gate_w):
    """Host router, replicating reference: softmax fp32 -> top-2 (ties to
    lower index, like lax.top_k) -> renormalize."""
    x = np.asarray(hidden_states, np.float32)
    logits = x @ np.asarray(gate_w, np.float32)
    logits = logits - logits.max(-1, keepdims=True)
    p = np.exp(logits)
    p /= p.sum(-1, keepdims=True)
    idx = np.argsort(-p, axis=-1, kind="stable")[:, :TOP_K]
    tw = np.take_along_axis(p, idx, -1)
    tw = (tw / tw.sum(-1, keepdims=True)).astype(np.float32)
    toks, wts = [], []
    for e in range(E):
        sel = idx == e                      # [T, K]; <=1 hit per token
        t = np.nonzero(sel.any(1))[0]
        w = np.where(sel[:, 0], tw[:, 0], tw[:, 1])[t]
        toks.append(t)
        wts.append(w.astype(np.float32))
    cap = max(len(t) for t in toks)
    C = max(((cap + 31) // 32) * 32, P)     # capacity, 32-aligned
    return toks, wts, C


def _chunks(C):
    """Split C tokens into equal-ish 32-aligned PSUM chunks of <=512."""
    n = -(-C // 512)
    s = ((-(-C // n) + 31) // 32) * 32
    bounds = []
    c0 = 0
    while c0 < C:
        c1 = min(c0 + s, C)
        bounds.append((c0, c1))
        c0 = c1
    return s, bounds


def build(C):
    MT = -(-C // P)                     # phase-B m-tiles (last may be partial)
    MS = [P] * (C // P) + ([C % P] if C % P else [])
    s, chunks = _chunks(C)

    nc = bacc.Bacc("TRN2", target_bir_lowering=False, debug=False,
                   num_devices=E)
    xg = nc.dram_tensor("xg", [P, KH, C], BF16, kind="ExternalInput")
    w13 = nc.dram_tensor("w13", [P, KF, 2, KH, P], BF16, kind="ExternalInput")
    w2d = nc.dram_tensor("w2d", [P, KF, H], BF16, kind="ExternalInput")
    cwd = nc.dram_tensor("cwd", [P, MT], F32, kind="ExternalInput")
    oute = nc.dram_tensor("oute", [C, H], F32, kind="ExternalOutput")

    with tile.TileContext(nc) as tc:
        with (
            tc.tile_pool(name="big", bufs=1) as big,
            tc.tile_pool(name="small", bufs=1) as small,
            tc.tile_pool(name="wpool", bufs=3) as wpool,
            tc.tile_pool(name="evac", bufs=4) as evac,
        ):
            # token shards: one tile per PSUM chunk so compute starts on
            # chunk 0 as soon as its DMA lands; chunk 0 is split across two
            # queues (per-queue DMA BW ~116GB/s is the startup bottleneck)
            xg_s = []
            for i, (c0, c1) in enumerate(chunks):
                xt = big.tile([P, KH, c1 - c0], BF16, name=f"xg{i}")
                if i == 0:
                    nc.gpsimd.dma_start(out=xt[:, :KH // 2],
                                        in_=xg.ap()[:, :KH // 2, c0:c1])
                    nc.sync.dma_start(out=xt[:, KH // 2:],
                                      in_=xg.ap()[:, KH // 2:, c0:c1])
                else:
                    nc.sync.dma_start(out=xt[:], in_=xg.ap()[:, :, c0:c1])
                xg_s.append(xt)
            cw_s = small.tile([P, MT], F32)
            nc.sync.dma_start(out=cw_s[:], in_=cwd.ap())
            # PE p-state warm-up: dummy accumulating matmuls on a zeroed tile
            # keep the Tensor engine busy (and ramping to 2.4GHz) while the
            # first xg/wf DMAs land
            warm_sb = small.tile([P, 256], BF16)
            nc.gpsimd.memset(warm_sb[:], 0.0)
            # w2 resident in SBUF (4MB bf16); streamed per k-slice inside the
            # phase A loop so it does not contend with the first wf/xg DMAs
            w2_s = big.tile([P, KF, H], BF16)
            inter = big.tile([P, KF, C], BF16)  # inter[f%P, f//P, tok]

            # Phase A: inter[f, t] = silu(w1.T x)[f, t] * (w3.T x)[f, t]
            with tc.tile_pool(name="psA", bufs=2, space=PSUM) as psA:
                warm = psA.tile([P, 256], F32, tag="warm", name="warm", bufs=1)
                for g in range(2):
                    for i in range(7):
                        nc.tensor.matmul(warm[:], lhsT=warm_sb[:, :P],
                                         rhs=warm_sb[:],
                                         start=(i == 0), stop=(i == 6))
                for f in range(KF):
                    wf = wpool.tile([P, 2, KH, P], BF16, tag="wf", name="wf",
                                    bufs=3)
                    if f == 0:
                        # spread the critical first tile over all 3 DMA queues
                        nc.scalar.dma_start(out=wf[:, 0],
                                            in_=w13.ap()[:, f, 0])
                        nc.gpsimd.dma_start(out=wf[:, 1, :KH // 2],
                                            in_=w13.ap()[:, f, 1, :KH // 2])
                        nc.sync.dma_start(out=wf[:, 1, KH // 2:],
                                          in_=w13.ap()[:, f, 1, KH // 2:])
                    else:
                        nc.gpsimd.dma_start(out=wf[:, 0], in_=w13.ap()[:, f, 0])
                        nc.scalar.dma_start(out=wf[:, 1], in_=w13.ap()[:, f, 1])
                    nc.sync.dma_start(out=w2_s[:, f, :], in_=w2d.ap()[:, f, :])
                    for i, (c0, c1) in enumerate(chunks):
                        w = c1 - c0
                        ps1 = psA.tile([P, s], F32, tag="ps1", name="ps1")
                        for k in range(KH):
                            nc.tensor.matmul(ps1[:, :w], lhsT=wf[:, 0, k, :],
                                             rhs=xg_s[i][:, k, :],
                                             start=(k == 0), stop=(k == KH - 1))
                        ps3 = psA.tile([P, s], F32, tag="ps3", name="ps3")
                        for k in range(KH):
                            nc.tensor.matmul(ps3[:, :w], lhsT=wf[:, 1, k, :],
                                             rhs=xg_s[i][:, k, :],
                                             start=(k == 0), stop=(k == KH - 1))
                        sil = evac.tile([P, s], F32, tag="sil", name="sil")
                        nc.scalar.activation(sil[:, :w], ps1[:, :w],
                                             mybir.ActivationFunctionType.Silu)
                        nc.vector.tensor_tensor(inter[:, f, c0:c1],
                                                sil[:, :w], ps3[:, :w],
                                                op=mybir.AluOpType.mult)

            # Phase B: oute[t, :] = cw[t] * (inter.T @ w2)[t, :]
            # m-outer so PSUM (2 banks per m-tile, double-buffered) frees as
            # soon as that m-tile's evac runs -- no barrier between m-tiles
            with tc.tile_pool(name="psB", bufs=2, space=PSUM) as psB:
                for m, ms in enumerate(MS):
                    pss = [psB.tile([P, 512], F32, tag=f"psb{n}",
                                    name=f"psb{n}") for n in range(H // 512)]
                    for k in range(KF):
                        for n in range(H // 512):
                            nc.tensor.matmul(
                                pss[n][:ms, :],
                                lhsT=inter[:, k, m * P:m * P + ms],
                                rhs=w2_s[:, k, n * 512:(n + 1) * 512],
                                start=(k == 0), stop=(k == KF - 1))
                    for n in range(H // 512):
                        o = evac.tile([P, 512], F32, tag="o", name="o")
                        nc.vector.tensor_scalar_mul(o[:ms, :], pss[n][:ms, :],
                                                    cw_s[:ms, m:m + 1])
                        eng = nc.sync if (2 * m + n) % 2 == 0 else nc.gpsimd
                        eng.dma_start(
                            out=oute.ap()[m * P:m * P + ms,
                                          n * 512:(n + 1) * 512],
                            in_=o[:ms, :])
    nc.compile()
    return nc


def kernel(hidden_states, gate_w, w1, w2, w3):
    in_maps = make_in_maps(hidden_states, gate_w, w1, w2, w3)
    nc = _NC_CACHE["nc"]
    res = run_bass_kernel_spmd(nc, in_maps, core_ids=list(range(E)),
                               trace=False)
    return assemble(res.results)


def make_in_maps(hidden_states, gate_w, w1, w2, w3):
    toks, wts, C = _route(hidden_states, gate_w)
    _NC_CACHE["route"] = (toks, wts, C)
    if "nc" not in _NC_CACHE or _NC_CACHE.get("C") != C:
        _NC_CACHE["nc"] = build(C)
        _NC_CACHE["C"] = C
    MT = -(-C // P)
    x = np.asarray(hidden_states, np.float32)
    in_maps = []
    for e in range(E):
        n_e = len(toks[e])
        xpad = np.zeros((C, H), np.float32)
        xpad[:n_e] = x[toks[e]]
        xgn = np.ascontiguousarray(
            xpad.reshape(C, KH, P).transpose(2, 1, 0).astype(BF16NP))
        w1r = np.asarray(w1[e], np.float32).reshape(KH, P, KF, P)
        w3r = np.asarray(w3[e], np.float32).reshape(KH, P, KF, P)
        w13n = np.ascontiguousarray(
            np.stack([w1r, w3r], 0).transpose(2, 3, 0, 1, 4).astype(BF16NP))
        w2n = np.ascontiguousarray(
            np.asarray(w2[e], np.float32).reshape(KF, P, H)
            .transpose(1, 0, 2).astype(BF16NP))
        wpad = np.zeros(MT * P, np.float32)
        wpad[:n_e] = wts[e]
        cwn = np.ascontiguousarray(wpad.reshape(MT, P).T)
        in_maps.append({"xg": xgn, "w13": w13n, "w2d": w2n, "cwd": cwn})
    return in_maps


def assemble(results):
    toks, _, _ = _NC_CACHE["route"]
    out = np.zeros((T, H), np.float32)
    for e in range(E):
        out[toks[e]] += results[e]["oute"][:len(toks[e])]
    return out


# revision 12
# speedup vs baseline: 1.1545x; 1.0043x over previous
"""Mixtral-style MoE block (T=2048, H=1024, F=2048, E=8, top-2) on 8 trn2
NeuronCores.

Expert-parallel with host-side dispatch: the router (a [2048,8] matmul +
softmax + top-2) is computed on host in fp32, and each core receives only
the tokens routed to its expert, capacity-padded to C (= max expert load
rounded up). The core computes its expert's SwiGLU FFN in bf16 (fp32 PSUM
accumulation), scales rows by the renormalized top-2 combine weight, and
writes its [C, H] partial; the host scatter-adds the two partials per
token. No collectives. bf16 halves both Tensor rows and HBM bytes vs
fp32r; sparse dispatch cuts matmul work 2048/C (~3.6x) vs dense.
"""
import numpy as np
import ml_dtypes

try:
    import concourse  # noqa: F401
except ImportError:  # pragma: no cover
    import sys
    sys.path.insert(0, "/opt/trn_rl_repo")

from concourse import mybir, bacc
import concourse.tile as tile
from concourse.bass_utils import run_bass_kernel_spmd

T, H, F, E, TOP_K = 2048, 1024, 2048, 8, 2
P = 128
KH = H // P          # 8 k-tiles over H
KF = F // P          # 16 f-tiles over F
F32 = mybir.dt.float32
BF16 = mybir.dt.bfloat16
BF16NP = ml_dtypes.bfloat16
PSUM = "PSUM"

_NC_CACHE = {}


def _route(hidden_states, gate_w):
    """Host router, replicating reference: softmax fp32 -> top-2 (ties to
    lower index, like lax.top_k) -> renormalize."""
    x = np.asarray(hidden_states, np.float32)
    logits = x @ np.asarray(gate_w, np.float32)
    logits = logits - logits.max(-1, keepdims=True)
    p = np.exp(logits)
    p /= p.sum(-1, keepdims=True)
    idx = np.argsort(-p, axis=-1, kind="stable")[:, :TOP_K]
    tw = np.take_along_axis(p, idx, -1)
    tw = (tw / tw.sum(-1, keepdims=True)).astype(np.float32)
    toks, wts = [], []
    for e in range(E):
        sel = idx == e                      # [T, K]; <=1 hit per token
        t = np.nonzero(sel.any(1))[0]
        w = np.where(sel[:, 0], tw[:, 0], tw[:, 1])[t]
        toks.append(t)
        wts.append(w.astype(np.float32))
    cap = max(len(t) for t in toks)
    C = max(((cap + 7) // 8) * 8, P)        # capacity, 8-aligned
    return toks, wts, C


def _chunks(C):
    """Split C tokens into up to 3 equal-ish 8-aligned PSUM chunks; small
    chunks let compute start as soon as the first lands (PSUM budget:
    2*nchunks+1 banks <= 8)."""
    n = 3 if C >= 384 else (2 if C >= 256 else 1)
    s = ((-(-C // n) + 7) // 8) * 8
    bounds = []
    c0 = 0
    while c0 < C:
        c1 = min(c0 + s, C)
        bounds.append((c0, c1))
        c0 = c1
    return s, bounds


def build(C):
    MT = -(-C // P)                     # phase-B m-tiles (last may be partial)
    MS = [P] * (C // P) + ([C % P] if C % P else [])
    s, chunks = _chunks(C)

    nc = bacc.Bacc("TRN2", target_bir_lowering=False, debug=False,
                   num_devices=E)
    xg = nc.dram_tensor("xg", [P, KH, C], BF16, kind="ExternalInput")
    w13 = nc.dram_tensor("w13", [P, KF, 2, KH, P], BF16, kind="ExternalInput")
    w2d = nc.dram_tensor("w2d", [P, KF, H], BF16, kind="ExternalInput")
    cwd = nc.dram_tensor("cwd", [P, MT], F32, kind="ExternalInput")
    oute = nc.dram_tensor("oute", [C, H], F32, kind="ExternalOutput")

    with tile.TileContext(nc) as tc:
        with (
            tc.tile_pool(name="big", bufs=1) as big,
            tc.tile_pool(name="small", bufs=1) as small,
            tc.tile_pool(name="wpool", bufs=3) as wpool,
            tc.tile_pool(name="evac", bufs=4) as evac,
        ):
            # PE p-state warm-up input: memset on the otherwise-idle Vector
            # engine so it does not queue behind DMA issues
            warm_sb = small.tile([P, 256], BF16)
            nc.vector.memset(warm_sb[:], 0.0)
            # token shards: one tile per PSUM chunk so compute starts on
            # chunk 0 as soon as its DMA lands; every chunk is split across
            # the gpsimd+sync queues (per-queue DMA BW is the startup
            # bottleneck, ~55-115GB/s)
            xg_s = []
            for i, (c0, c1) in enumerate(chunks):
                xt = big.tile([P, KH, c1 - c0], BF16, name=f"xg{i}")
                nc.gpsimd.dma_start(out=xt[:, :KH // 2],
                                    in_=xg.ap()[:, :KH // 2, c0:c1])
                nc.sync.dma_start(out=xt[:, KH // 2:],
                                  in_=xg.ap()[:, KH // 2:, c0:c1])
                xg_s.append(xt)
            cw_s = small.tile([P, MT], F32)
            nc.sync.dma_start(out=cw_s[:], in_=cwd.ap())
            # w2 resident in SBUF (4MB bf16); streamed per k-slice inside the
            # phase A loop so it does not contend with the first wf/xg DMAs
            w2_s = big.tile([P, KF, H], BF16)
            inter = big.tile([P, KF, C], BF16)  # inter[f%P, f//P, tok]

            # Phase A: inter[f, t] = silu(w1.T x)[f, t] * (w3.T x)[f, t]
            # per f-tile: all w1 chunk-groups first, then all w3 groups, so
            # the w3 weight half may arrive ~2us later than the w1 half
            with tc.tile_pool(name="psA", bufs=1, space=PSUM) as psA:
                # dummy accumulating matmuls keep the Tensor engine busy (and
                # its p-state ramping toward 2.4GHz) while the first DMAs land
                warm = psA.tile([P, 256], F32, tag="warm", name="warm", bufs=1)
                for i in range(8):
                    nc.tensor.matmul(warm[:], lhsT=warm_sb[:, :P],
                                     rhs=warm_sb[:],
                                     start=(i == 0), stop=(i == 7))
                for f in range(KF):
                    wf = wpool.tile([P, 2, KH, P], BF16, tag="wf", name="wf",
                                    bufs=3)
                    if f == 0:
                        # whole first tile on the scalar queue (gpsimd/sync
                        # carry the xg chunks)
                        nc.scalar.dma_start(out=wf[:], in_=w13.ap()[:, f])
                    else:
                        nc.gpsimd.dma_start(out=wf[:, 0], in_=w13.ap()[:, f, 0])
                        nc.scalar.dma_start(out=wf[:, 1], in_=w13.ap()[:, f, 1])
                    nc.sync.dma_start(out=w2_s[:, f, :], in_=w2d.ap()[:, f, :])
                    pss = []
                    for i, (c0, c1) in enumerate(chunks):
                        w = c1 - c0
                        ps1 = psA.tile([P, s], F32, tag=f"ps1_{i}",
                                       name=f"ps1_{i}")
                        for k in range(KH):
                            nc.tensor.matmul(ps1[:, :w], lhsT=wf[:, 0, k, :],
                                             rhs=xg_s[i][:, k, :],
                                             start=(k == 0), stop=(k == KH - 1))
                        sil = evac.tile([P, s], F32, tag=f"sil_{i}",
                                        name=f"sil_{i}", bufs=1)
                        nc.scalar.activation(sil[:, :w], ps1[:, :w],
                                             mybir.ActivationFunctionType.Silu)
                        pss.append(sil)
                    for i, (c0, c1) in enumerate(chunks):
                        w = c1 - c0
                        ps3 = psA.tile([P, s], F32, tag=f"ps3_{i}",
                                       name=f"ps3_{i}")
                        for k in range(KH):
                            nc.tensor.matmul(ps3[:, :w], lhsT=wf[:, 1, k, :],
                                             rhs=xg_s[i][:, k, :],
                                             start=(k == 0), stop=(k == KH - 1))
                        nc.vector.tensor_tensor(inter[:, f, c0:c1],
                                                pss[i][:, :w], ps3[:, :w],
                                                op=mybir.AluOpType.mult)

            # Phase B: oute[t, :] = cw[t] * (inter.T @ w2)[t, :]
            # (m, n)-outer, k-inner: each [ms, 512] PSUM group evacs while
            # the next group's matmuls run, so the final evac+DMA tail only
            # trails the very last 16-matmul group
            with tc.tile_pool(name="psB", bufs=2, space=PSUM) as psB:
                for m, ms in enumerate(MS):
                    for n in range(H // 512):
                        ps = psB.tile([P, 512], F32, tag=f"psb{n}",
                                      name=f"psb{n}")
                        for k in range(KF):
                            nc.tensor.matmul(
                                ps[:ms, :],
                                lhsT=inter[:, k, m * P:m * P + ms],
                                rhs=w2_s[:, k, n * 512:(n + 1) * 512],
                                start=(k == 0), stop=(k == KF - 1))
                        o = evac.tile([P, 512], F32, tag="o", name="o")
                        nc.vector.tensor_scalar_mul(o[:ms, :], ps[:ms, :],
                                                    cw_s[:ms, m:m + 1])
                        eng = nc.sync if (2 * m + n) % 2 == 0 else nc.gpsimd
                        eng.dma_start(
                            out=oute.ap()[m * P:m * P + ms,
                                          n * 512:(n + 1) * 512],
                            in_=o[:ms, :])
    nc.compile()
    return nc


def kernel(hidden_states, gate_w, w1, w2, w3):
    in_maps = make_in_maps(hidden_states, gate_w, w1, w2, w3)
    nc = _NC_CACHE["nc"]
    res = run_bass_kernel_spmd(nc, in_maps, core_ids=list(range(E)),
                               trace=False)
    return assemble(res.results)


def make_in_maps(hidden_states, gate_w, w1, w2, w3):
    toks, wts, C = _route(hidden_states, gate_w)
    _NC_CACHE["route"] = (toks, wts, C)
    if "nc" not in _NC_CACHE or _NC_CACHE.get("C") != C:
        _NC_CACHE["nc"] = build(C)
        _NC_CACHE["C"] = C
    MT = -(-C // P)
    x = np.asarray(hidden_states, np.float32)
    in_maps = []
    for e in range(E):
        n_e = len(toks[e])
        xpad = np.zeros((C, H), np.float32)
        xpad[:n_e] = x[toks[e]]
        xgn = np.ascontiguousarray(
            xpad.reshape(C, KH, P).transpose(2, 1, 0).astype(BF16NP))
        w1r = np.asarray(w1[e], np.float32).reshape(KH, P, KF, P)
        w3r = np.asarray(w3[e], np.float32).reshape(KH, P, KF, P)
        w13n = np.ascontiguousarray(
            np.stack([w1r, w3r], 0).transpose(2, 3, 0, 1, 4).astype(BF16NP))
        w2n = np.ascontiguousarray(
            np.asarray(w2[e], np.float32).reshape(KF, P, H)
            .transpose(1, 0, 2).astype(BF16NP))
        wpad = np.zeros(MT * P, np.float32)
        wpad[:n_e] = wts[e]
        cwn = np.ascontiguousarray(wpad.reshape(MT, P).T)
        in_maps.append({"xg": xgn, "w13": w13n, "w2d": w2n, "cwd": cwn})
    return in_maps


def assemble(results):
    toks, _, _ = _NC_CACHE["route"]
    out = np.zeros((T, H), np.float32)
    for e in range(E):
        out[toks[e]] += results[e]["oute"][:len(toks[e])]
    return out
